# revision 18
# baseline (speedup 1.0000x reference)
"""2-layer GAT (PyG GATConv style) distributed across 8 TRN2 NeuronCores.

Sharding: nodes partitioned into 8 contiguous destination blocks (one per
core).  The Q7/SWDGE descriptor generation for per-edge dma_gathers is the
machine bottleneck, so the design minimizes gather descriptors:

  - Slot-level edge layout (no Q-slot bins): slot (p, c) of a 128-dst group
    holds exactly one edge; per-(group, half) gather counts are truncated to
    the real edge count (padded to 16), so descriptors ~= edges.
  - Self-loops never enter the gather path: their contribution is applied
    densely in the per-group epilogue from SBUF-resident own-block rows.
  - No per-bin dst-logit gather: ald[dst] is delivered to slots by C small
    TensorE matmuls per group (lhsT = transposed 0/1 dst-selector built on
    DVE from a partition-broadcast flat dst image).
  - Attention logits als/ald are computed by widening the xp matmul rhs with
    precomputed W@a columns (no DVE reductions).

Node tables are bf16 rows [xp | als] gathered by src (768B pitch layer 1,
256B pitch layer 2), split into two sub-tables (local row < HL) so gather
indices fit int16; each sub-table is AllGathered separately so collectives
overlap compute.  Host preprocessing is index-only; all float math runs on
device.
"""

import math
from contextlib import ExitStack

import numpy as np
import ml_dtypes

import concourse.bass as bass
import concourse.tile as tile
from concourse import bacc, mybir
from concourse.bass_utils import run_bass_kernel_spmd
from concourse.masks import make_identity

F32 = mybir.dt.float32
BF16 = mybir.dt.bfloat16
I16 = mybir.dt.int16
AF = mybir.ActivationFunctionType
OP = mybir.AluOpType

P = 128
NEG_SLOPE = 0.2
HL = 3200             # local-row split: half A = rows [0, HL) of each block


def dma_gather_raw(gp, out_ap, in_ap, idxs_ap, num_idxs, elem_size,
                   elem_step, single_packet=None):
    """BassGpSimd.dma_gather minus the payload%256 assert (pitch must still
    be a 256B multiple; verified on HW that arbitrary payload works)."""
    from concourse._compat import exact_div
    assert idxs_ap.dtype == mybir.dt.int16
    assert in_ap.dtype == out_ap.dtype
    stride_bytes = elem_step * mybir.dt.size(in_ap.dtype)
    stride_bytes_256 = exact_div(stride_bytes, 256)
    assert stride_bytes_256 < 256
    _in_ap = gp.lower_ap_dma(in_ap, for_custom_bir_dma=True)
    _idxs_ap = gp.lower_ap(idxs_ap)
    _out_ap = gp.lower_ap(out_ap)
    return gp.add_instruction(
        mybir.InstDMAGatherAnt(
            name=gp.bass.get_next_instruction_name(),
            ins=[*_in_ap, _idxs_ap, gp.lower_val_access(gp.to_reg(num_idxs))],
            outs=[_out_ap],
            transpose=False,
            num_idxs=num_idxs,
            elem_size=elem_size,
            stride_bytes_256=stride_bytes_256,
            gen_mode=0,
            single_packet=(False if single_packet is None
                           else single_packet),
            queue_num=0,
            sbuf_tokens_per_rank=0,
            sbuf_free_dim_per_rank=0,
            sbuf_free_dim_pad_per_rank=0,
            sbuf_byte_offset=0,
        ))


class Dims:
    def __init__(self, N, E, n_cores, H1=4, C1=64, H2=1, C2=16, F_in=256):
        self.N, self.E, self.NC = N, E, n_cores
        self.F_in = F_in
        self.H1, self.C1, self.H2, self.C2 = H1, C1, H2, C2
        self.D1 = H1 * C1          # 256
        self.D2 = H2 * C2          # 16
        self.B = N // n_cores      # 6250
        self.G = math.ceil(self.B / P)   # 49
        self.HLB = self.B - HL     # B-half rows per block (3050)
        self.NA = n_cores * HL     # 25600
        self.NB = n_cores * self.HLB   # 24400
        self.R1 = self.D1 + self.H1    # 260: [xp1 | als1]
        self.T1 = 384                  # layer-1 table pitch (768B bf16)
        self.R2 = self.D2 + self.H2    # 17: [xp2 | als2]
        self.T2 = 128                  # layer-2 table pitch (256B bf16)
        # set by host_prep (shared across cores; per-group capacities)
        self.nA = None   # [G] gather counts half A (mult of 16)
        self.nB = None
        self.CA = None   # [G] column counts half A
        self.CB = None
        self.CMAX = None
        self.SAmax = None  # idx-image cols per group (A)
        self.SBmax = None


def _wrap_idx16(flat, cols):
    """index list -> [128, cols] int16 image (16-partition wrap, replicated
    for the 8 Q7 cores). flat may be shorter than 16*cols (zero pad)."""
    a = np.zeros((16, cols), np.int16)
    n = len(flat)
    i = np.arange(n)
    a[i % 16, i // 16] = flat
    return np.tile(a, (8, 1))


def host_prep(dims: Dims, edge_index: np.ndarray):
    """Index-only preprocessing. Self-loops are NOT added here (handled
    densely on device). Edges are bucketed per (core, dst-group, src-half)
    and assigned slots (p, c) in arrival order: slot i -> (c=i//128,
    p=i%128)."""
    N, NC, B, G = dims.N, dims.NC, dims.B, dims.G
    src = edge_index[0].astype(np.int64)
    dst = edge_index[1].astype(np.int64)

    # per (core, group, half) edge lists (table-local row ids + local dst)
    counts = np.zeros((NC, G, 2), np.int64)
    lists = [[None] * G for _ in range(NC)]
    for k in range(NC):
        m = (dst >= k * B) & (dst < (k + 1) * B)
        s_k = src[m]
        d_k = dst[m] - k * B
        half = ((s_k % B) >= HL).astype(np.int64)
        # table-local row id
        rowA = (s_k // B) * HL + (s_k % B)
        rowB = (s_k // B) * dims.HLB + (s_k % B) - HL
        row = np.where(half == 0, rowA, rowB)
        g = d_k // P
        p_loc = d_k % P
        order = np.lexsort((row, g, half))  # group by (half, g)
        s_o, g_o, h_o, r_o, p_o = (s_k[order], g[order], half[order],
                                   row[order], p_loc[order])
        for gg in range(G):
            for hh in range(2):
                mm = (g_o == gg) & (h_o == hh)
                if lists[k][gg] is None:
                    lists[k][gg] = [None, None]
                lists[k][gg][hh] = (r_o[mm], p_o[mm])
                counts[k, gg, hh] = mm.sum()

    def pad16(x):
        return ((x + 15) // 16) * 16

    nA = pad16(counts[:, :, 0].max(axis=0))
    nB = pad16(counts[:, :, 1].max(axis=0))
    nA = np.maximum(nA, 16)
    nB = np.maximum(nB, 16)
    CA = np.ceil(nA / P).astype(np.int64)
    CB = np.ceil(nB / P).astype(np.int64)
    dims.nA, dims.nB = [int(v) for v in nA], [int(v) for v in nB]
    dims.CA, dims.CB = [int(v) for v in CA], [int(v) for v in CB]
    dims.CMAX = int((CA + CB).max())
    dims.SAmax = int(nA.max() // 16)
    dims.SBmax = int(nB.max() // 16)

    CMAX = dims.CMAX
    SW = dims.SAmax + dims.SBmax
    per_core = []
    for k in range(NC):
        idx = np.zeros((P, G * SW), np.int16)
        dstl = np.full((P, G * CMAX), -1.0, np.float32)
        dstlF = np.full((G, CMAX * P), -1.0, np.float32)
        for g in range(G):
            C_g = dims.CA[g] + dims.CB[g]
            dflat = np.full(C_g * P, -1.0, np.float32)
            for hh in range(2):
                rows, ps = lists[k][g][hh]
                n = len(rows)
                coff = 0 if hh == 0 else dims.CA[g]
                i = np.arange(n)
                c = i // P + coff
                p = i % P
                # slot (p, c) holds an edge whose LOCAL dst (d % 128) is ps
                dstl[p, g * CMAX + c] = ps
                dflat[c * P + p] = ps
                cap = dims.nA[g] if hh == 0 else dims.nB[g]
                scols = dims.SAmax if hh == 0 else dims.SBmax
                soff = g * SW + (0 if hh == 0 else dims.SAmax)
                idx[:, soff:soff + scols] = _wrap_idx16(rows, scols)
            dstlF[g, 0:C_g * P] = dflat
        per_core.append(dict(
            idx=idx,
            dstl=np.ascontiguousarray(dstl).astype(ml_dtypes.bfloat16),
            dstlF=np.ascontiguousarray(dstlF).astype(ml_dtypes.bfloat16),
        ))
    return per_core


def build_program(dims: Dims):
    N, NC, B, G = dims.N, dims.NC, dims.B, dims.G
    F_in, D1, D2, H1, H2 = dims.F_in, dims.D1, dims.D2, dims.H1, dims.H2
    C1 = dims.C1
    R1, T1, R2, T2 = dims.R1, dims.T1, dims.R2, dims.T2
    NA, NB, HLB = dims.NA, dims.NB, dims.HLB
    CMAX = dims.CMAX
    SW = dims.SAmax + dims.SBmax
    KF = F_in // P     # 2
    KD = D1 // P       # 2
    W1C = D1 + 2 * H1  # w1 rhs width: [W1 | va1s | va1d] = 264
    W2C = D2 + 2 * H2  # 18

    nc = bacc.Bacc("TRN2", target_bir_lowering=False, debug=False,
                   enable_asserts=False, num_devices=NC)

    xT = nc.dram_tensor("xT", [F_in, B], F32, kind="ExternalInput")
    W1 = nc.dram_tensor("W1", [F_in, D1], F32, kind="ExternalInput")
    a1s = nc.dram_tensor("a1s", [D1], F32, kind="ExternalInput")
    a1d = nc.dram_tensor("a1d", [D1], F32, kind="ExternalInput")
    b1 = nc.dram_tensor("b1", [D1], F32, kind="ExternalInput")
    W2 = nc.dram_tensor("W2", [D1, D2], F32, kind="ExternalInput")
    a2s = nc.dram_tensor("a2s", [D2], F32, kind="ExternalInput")
    a2d = nc.dram_tensor("a2d", [D2], F32, kind="ExternalInput")
    b2 = nc.dram_tensor("b2", [D2], F32, kind="ExternalInput")
    idx = nc.dram_tensor("idx", [P, G * SW], I16, kind="ExternalInput")
    dstl = nc.dram_tensor("dstl", [P, G * CMAX], BF16, kind="ExternalInput")
    dstlF = nc.dram_tensor("dstlF", [G, CMAX * P], BF16,
                           kind="ExternalInput")
    out2 = nc.dram_tensor("out2", [B, D2], F32, kind="ExternalOutput")

    t1A_loc = nc.dram_tensor("t1A_loc", [HL, T1], BF16)
    t1B_loc = nc.dram_tensor("t1B_loc", [HLB, T1], BF16)
    t1A_full = nc.dram_tensor("t1A_full", [NA, T1], BF16, addr_space="Shared")
    t1B_full = nc.dram_tensor("t1B_full", [NB, T1], BF16, addr_space="Shared")
    t2A_loc = nc.dram_tensor("t2A_loc", [HL, T2], BF16)
    t2B_loc = nc.dram_tensor("t2B_loc", [HLB, T2], BF16)
    t2A_full = nc.dram_tensor("t2A_full", [NA, T2], BF16, addr_space="Shared")
    t2B_full = nc.dram_tensor("t2B_full", [NB, T2], BF16, addr_space="Shared")

    rg = [list(range(NC))]

    with tile.TileContext(nc) as tc, ExitStack() as ctx:
        const = ctx.enter_context(tc.tile_pool(name="const", bufs=1))
        ictx = ExitStack()
        cpsum = ictx.enter_context(tc.tile_pool(name="cpsum", bufs=1,
                                                space="PSUM"))
        itmp = ictx.enter_context(tc.tile_pool(name="itmp", bufs=1))

        iota_i = itmp.tile([P, P], mybir.dt.int32, tag="iota_i")
        nc.gpsimd.iota(iota_i[:], pattern=[[1, P]], base=0,
                       channel_multiplier=0)
        iota_bf = const.tile([P, P], BF16, tag="iota_bf")
        nc.vector.tensor_copy(iota_bf[:], iota_i[:])
        # partition-index iota, constant along free dim
        iotaP_i = itmp.tile([P, CMAX * P], mybir.dt.int32, tag="iotaP_i")
        nc.gpsimd.iota(iotaP_i[:], pattern=[[0, CMAX * P]], base=0,
                       channel_multiplier=1)
        iotaP = const.tile([P, CMAX * P], BF16, tag="iotaP")
        nc.vector.tensor_copy(iotaP[:], iotaP_i[:])
        ident = const.tile([P, P], BF16, tag="ident")
        make_identity(nc, ident[:])

        # weights rhs: [W1 | va1s | va1d], [W2 | va2s | va2d]
        w1sb = const.tile([P, KF, W1C], BF16, tag="w1sb")
        for c in range(KF):
            nc.gpsimd.dma_start(out=w1sb[:, c, 0:D1],
                                in_=W1[c * P:(c + 1) * P, :])
        w2sb = const.tile([P, KD, W2C], BF16, tag="w2sb")
        for c in range(KD):
            nc.gpsimd.dma_start(out=w2sb[:, c, 0:D2],
                                in_=W2[c * P:(c + 1) * P, :])

        ones_row = const.tile([1, P], BF16, tag="ones_row")
        nc.vector.memset(ones_row[:], 1.0)
        ones_rowf = itmp.tile([1, P], F32, tag="ones_rowf")
        nc.vector.memset(ones_rowf[:], 1.0)

        def replicate(vec_ap, X, tag, pool):
            vrow = itmp.tile([1, X], F32, tag=tag + "_row")
            nc.sync.dma_start(out=vrow[:], in_=vec_ap[None, :])
            pr = cpsum.tile([P, X], F32, tag="reppsum")
            nc.tensor.matmul(out=pr[:], lhsT=ones_rowf[:], rhs=vrow[:],
                             start=True, stop=True)
            rep = pool.tile([P, X], F32, tag=tag)
            nc.vector.tensor_copy(rep[:], pr[:])
            return rep

        a1s_r = replicate(a1s, D1, "a1s_r", itmp)
        a1d_r = replicate(a1d, D1, "a1d_r", itmp)
        b1_r = replicate(b1, D1, "b1_r", const)
        a2s_r = replicate(a2s, D2, "a2s_r", itmp)
        a2d_r = replicate(a2d, D2, "a2d_r", itmp)
        b2_r = replicate(b2, D2, "b2_r", const)

        slopeH = const.tile([P, H1], F32, tag="slopeH")
        nc.vector.memset(slopeH[:], NEG_SLOPE)
        zeroD = const.tile([P, D1], F32, tag="zeroD")
        nc.vector.memset(zeroD[:], 0.0)
        negoneD = const.tile([P, D1], F32, tag="negoneD")
        nc.vector.memset(negoneD[:], -1.0)

        # va1s[f, h] = sum_c W1[f, (c,h)] * a1s[(c,h)]   (features are
        # (c,h)-interleaved: head h at stride-H1 positions)
        for c in range(KF):
            for (vec, off) in ((a1s_r, D1), (a1d_r, D1 + H1)):
                tmpv = itmp.tile([P, D1], F32, tag="vamul")
                nc.vector.tensor_tensor(out=tmpv[:], in0=w1sb[:, c, 0:D1],
                                        in1=vec[:], op=OP.mult)
                vaf = itmp.tile([P, H1], F32, tag="vaf")
                nc.vector.tensor_reduce(
                    out=vaf[:],
                    in_=tmpv[:].rearrange("p (c h) -> p h c", h=H1),
                    axis=mybir.AxisListType.X, op=OP.add)
                nc.vector.tensor_copy(w1sb[:, c, off:off + H1], vaf[:])
        for c in range(KD):
            for (vec, off) in ((a2s_r, D2), (a2d_r, D2 + H2)):
                tmpv = itmp.tile([P, D2], F32, tag="vamul2")
                nc.vector.tensor_tensor(out=tmpv[:], in0=w2sb[:, c, 0:D2],
                                        in1=vec[:], op=OP.mult)
                vaf = itmp.tile([P, H2], F32, tag="vaf2")
                nc.vector.tensor_reduce(out=vaf[:], in_=tmpv[:],
                                        axis=mybir.AxisListType.X, op=OP.add)
                nc.vector.tensor_copy(w2sb[:, c, off:off + H2], vaf[:])

        idx_sb = const.tile([P, G * SW], I16, tag="idx_sb")
        nc.sync.dma_start(out=idx_sb[:], in_=idx[:, :])
        dstl_sb = const.tile([P, G * CMAX], BF16, tag="dstl_sb")
        nc.sync.dma_start(out=dstl_sb[:], in_=dstl[:, :])

        # SBUF-resident per-node rows of the own block:
        # layer 1: [xp1 (256) | als1 (4) | ald1 (4)]; layer 2: [xp2|als2|ald2]
        xp1_res = const.tile([P, G, W1C], BF16, tag="xp1_res")
        nc.vector.memset(xp1_res[:], 0.0)
        xp2_res = const.tile([P, G, W2C], BF16, tag="xp2_res")

        ictx.close()

        # ---- stage A: xp1 + logits for own block; fill t1 tables ----------
        actx = ExitStack()
        pa = actx.enter_context(tc.tile_pool(name="pa", bufs=3))
        pa_ps = actx.enter_context(tc.tile_pool(name="pa_ps", bufs=2,
                                                space="PSUM"))
        # whole x^T block resident in SBUF for stage A, loaded in chunks so
        # the first tiles' matmuls start early (sync f32 load + DVE cast)
        xpool = actx.enter_context(tc.tile_pool(name="xsb", bufs=1))
        xsb = xpool.tile([P, KF, B], BF16, tag="xsb")
        XCH = 8
        xbnd = [round(B * i / XCH) for i in range(XCH + 1)]
        for i in range(XCH):
            lo, hi = xbnd[i], xbnd[i + 1]
            xf = pa.tile([P, KF, (B + XCH - 1) // XCH + 1], F32, tag="xf")
            for c in range(KF):
                nc.sync.dma_start(out=xf[:, c, 0:hi - lo],
                                  in_=xT[c * P:(c + 1) * P, lo:hi])
            nc.vector.tensor_copy(xsb[:, :, lo:hi], xf[:, :, 0:hi - lo])
        for t in range(G):
            n0 = t * P
            nn = min(P, B - n0)
            ps_a = pa_ps.tile([P, W1C], F32, tag="ps_a")
            for c in range(KF):
                nc.tensor.matmul(out=ps_a[:nn, :], lhsT=xsb[:, c, n0:n0 + nn],
                                 rhs=w1sb[:, c, :],
                                 start=(c == 0), stop=(c == KF - 1))
            nc.scalar.copy(out=xp1_res[:nn, t, :], in_=ps_a[:nn, :])
            wq = nc.sync if t % 2 == 0 else nc.scalar
            if t < HL // P:
                wq.dma_start(out=t1A_loc[n0:n0 + nn, 0:R1],
                             in_=xp1_res[:nn, t, 0:R1])
            else:
                m0 = n0 - HL
                wq.dma_start(out=t1B_loc[m0:m0 + nn, 0:R1],
                             in_=xp1_res[:nn, t, 0:R1])
            if t == HL // P - 1:
                nc.gpsimd.collective_compute(
                    "AllGather", OP.bypass, replica_groups=rg,
                    ins=[t1A_loc.ap()], outs=[t1A_full.ap()])
        nc.gpsimd.collective_compute(
            "AllGather", OP.bypass, replica_groups=rg,
            ins=[t1B_loc.ap()], outs=[t1B_full.ap()])
        actx.close()

        # ---- edge phase (shared structure for both layers) ----------------
        def edge_phase(layer, fctx):
            """layer 1: gathers t1 rows, aggregates, writes t2 tables.
            layer 2: gathers t2 rows, aggregates, writes log_softmax out."""
            R = R1 if layer == 1 else R2
            D = D1 if layer == 1 else D2
            H = H1 if layer == 1 else H2
            T = T1 if layer == 1 else T2
            tA = t1A_full if layer == 1 else t2A_full
            tB = t1B_full if layer == 1 else t2B_full
            res = xp1_res if layer == 1 else xp2_res

            PRE = 4
            pg = fctx.enter_context(tc.tile_pool(name=f"pg{layer}",
                                                 bufs=PRE + 1))
            pm = fctx.enter_context(tc.tile_pool(name=f"pm{layer}", bufs=2))
            pe = fctx.enter_context(tc.tile_pool(name=f"pe{layer}", bufs=3))
            pres = fctx.enter_context(tc.tile_pool(name=f"pres{layer}",
                                                   bufs=1))
            ps_pool = fctx.enter_context(tc.tile_pool(
                name=f"ps{layer}", bufs=2, space="PSUM"))
            psa_pool = fctx.enter_context(tc.tile_pool(
                name=f"psa{layer}", bufs=2, space="PSUM"))
            psr_pool = fctx.enter_context(tc.tile_pool(
                name=f"psr{layer}", bufs=2, space="PSUM"))
            if layer == 1:
                pt_ps = fctx.enter_context(tc.tile_pool(
                    name="pt_ps", bufs=1, space="PSUM"))

            # batched self-loop softmax numerators: exs_res[p, g, h]
            eps_all = pres.tile([P, G, H], F32, tag="eps_all")
            nc.vector.tensor_tensor(out=eps_all[:], in0=res[:, :, D:D + H],
                                    in1=res[:, :, D + H:D + 2 * H],
                                    op=OP.add)
            lr_all = pres.tile([P, G, H], F32, tag="lr_all")
            nc.vector.tensor_tensor(
                out=lr_all[:], in0=eps_all[:],
                in1=slopeH[:, None, 0:H].to_broadcast([P, G, H]), op=OP.mult)
            nc.vector.tensor_tensor(out=lr_all[:], in0=lr_all[:],
                                    in1=eps_all[:], op=OP.max)
            exs_res = pres.tile([P, G, H], F32, tag="exs_res")
            nc.scalar.activation(exs_res[:], lr_all[:], AF.Exp)
            if layer == 2:
                xs_res = pres.tile([P, G, D], F32, tag="xs_res")
                ssum_res = pres.tile([P, G], F32, tag="ssum_res")

            gat_tiles = {}

            def issue_A(g):
                CAg = dims.CA[g]
                nAg = dims.nA[g]
                gat = pg.tile([P, CMAX, R], BF16, tag="gat")
                if g <= PRE:
                    nc.vector.memset(gat[:], 0.0)
                dma_gather_raw(nc.gpsimd, gat[:, 0:CAg, :], tA[0:NA, 0:R],
                               idx_sb[:, g * SW:g * SW + nAg // 16],
                               nAg, R, T)
                gat_tiles[g] = gat

            def issue_B(g):
                CAg, CBg = dims.CA[g], dims.CB[g]
                nBg = dims.nB[g]
                gat = gat_tiles[g]
                dma_gather_raw(nc.gpsimd, gat[:, CAg:CAg + CBg, :],
                               tB[0:NB, 0:R],
                               idx_sb[:, g * SW + dims.SAmax:
                                      g * SW + dims.SAmax + nBg // 16],
                               nBg, R, T)

            for g in range(min(PRE, G)):
                issue_A(g)
            for g in range(min(PRE, G)):
                issue_B(g)

            for g in range(G):
                if g + PRE < G:
                    issue_A(g + PRE)
                    issue_B(g + PRE)
                w0 = g * P
                wn = min(P, B - w0)
                CAg, CBg = dims.CA[g], dims.CB[g]
                Cg = CAg + CBg
                gat = gat_tiles.pop(g)

                # transposed selector stT[d, c*128+p] = (dst(p,c) == d):
                # replicate the flat dst row across partitions via TensorE,
                # then one DVE compare against the partition-index iota.
                flrow = pe.tile([1, CMAX * P], BF16, tag="flrow")
                nc.sync.dma_start(out=flrow[0:1, 0:Cg * P],
                                  in_=dstlF[g:g + 1, 0:Cg * P])
                fl = flrow[0:1, 0:Cg * P]
                stT = pm.tile([P, CMAX * P], BF16, tag="stT")
                for o in range(0, Cg * P, 512):
                    w = min(512, Cg * P - o)
                    rep = psr_pool.tile([P, 512], F32, tag="rep")
                    nc.tensor.matmul(out=rep[:, 0:w], lhsT=ones_row[:],
                                     rhs=fl[:, o:o + w],
                                     start=True, stop=True)
                    nc.vector.tensor_tensor(
                        out=stT[:, o:o + w], in0=iotaP[:, o:o + w],
                        in1=rep[:, 0:w], op=OP.is_equal)
                # ald per slot via C small matmuls
                ps_ald = psa_pool.tile([P, CMAX * H], F32, tag="ps_ald")
                for c in range(Cg):
                    nc.tensor.matmul(
                        out=ps_ald[:, c * H:(c + 1) * H],
                        lhsT=stT[:, c * P:(c + 1) * P],
                        rhs=res[:, g, D + H:D + 2 * H],
                        start=True, stop=True)

                # ex = exp(leaky_relu(als[s] + ald[d]))
                ep = pe.tile([P, CMAX, H], F32, tag="ep")
                nc.vector.tensor_tensor(
                    out=ep[:, 0:Cg, :], in0=gat[:, 0:Cg, D:D + H],
                    in1=ps_ald[:].rearrange("p (c h) -> p c h",
                                            h=H)[:, 0:Cg, :],
                    op=OP.add)
                lr = pe.tile([P, CMAX, H], F32, tag="lr")
                nc.vector.tensor_tensor(
                    out=lr[:, 0:Cg, :], in0=ep[:, 0:Cg, :],
                    in1=slopeH[:, None, 0:H].to_broadcast([P, Cg, H]),
                    op=OP.mult)
                nc.vector.tensor_tensor(out=lr[:, 0:Cg, :],
                                        in0=lr[:, 0:Cg, :],
                                        in1=ep[:, 0:Cg, :], op=OP.max)
                msg = pm.tile([P, CMAX, R], BF16, tag="msg")
                nc.scalar.activation(msg[:, 0:Cg, D:D + H], lr[:, 0:Cg, :],
                                     AF.Exp)
                if layer == 1:
                    nc.vector.tensor_tensor(
                        out=msg[:, 0:Cg, 0:D].rearrange(
                            "p k (c h) -> p k c h", h=H),
                        in0=gat[:, 0:Cg, 0:D].rearrange(
                            "p k (c h) -> p k c h", h=H),
                        in1=msg[:, 0:Cg, D:D + H][:, :, None, :].to_broadcast(
                            [P, Cg, C1, H]),
                        op=OP.mult)
                else:
                    nc.vector.tensor_tensor(
                        out=msg[:, 0:Cg, 0:D], in0=gat[:, 0:Cg, 0:D],
                        in1=msg[:, 0:Cg, D:D + H].to_broadcast([P, Cg, D]),
                        op=OP.mult)

                # segment-sum matmuls: selector st[p, c, d]
                st = pm.tile([P, CMAX, P], BF16, tag="st")
                nc.vector.tensor_tensor(
                    out=st[:, 0:Cg, :],
                    in0=iota_bf[:, None, :].to_broadcast([P, Cg, P]),
                    in1=dstl_sb[:, g * CMAX:g * CMAX + Cg][:, :, None]
                        .to_broadcast([P, Cg, P]),
                    op=OP.is_equal)
                ps_g = ps_pool.tile([P, R], F32, tag="ps_g")
                for col in range(Cg):
                    nc.tensor.matmul(out=ps_g[:], lhsT=st[:, col, :],
                                     rhs=msg[:, col, :],
                                     start=(col == 0), stop=(col == Cg - 1))

                # epilogue: self-loop + alpha-normalize
                den = pe.tile([P, H], F32, tag="den")
                nc.vector.tensor_tensor(out=den[:], in0=ps_g[:, D:D + H],
                                        in1=exs_res[:, g, :], op=OP.add)
                rec = pe.tile([P, H], F32, tag="rec")
                nc.vector.reciprocal(rec[:], den[:])

                if layer == 1:
                    smsg = pe.tile([P, D], F32, tag="smsg")
                    nc.vector.tensor_tensor(
                        out=smsg[:].rearrange("p (c h) -> p c h", h=H),
                        in0=res[:, g, 0:D].rearrange("p (c h) -> p c h", h=H),
                        in1=exs_res[:, g, :][:, None, :].to_broadcast(
                            [P, C1, H]),
                        op=OP.mult)
                    num = pe.tile([P, D], F32, tag="num")
                    nc.vector.tensor_tensor(out=num[:], in0=ps_g[:, 0:D],
                                            in1=smsg[:], op=OP.add)
                    h1f = pg.tile([P, D], F32, tag="h1f")
                    nc.vector.tensor_tensor(
                        out=h1f[:].rearrange("p (c h) -> p c h", h=H),
                        in0=num[:].rearrange("p (c h) -> p c h", h=H),
                        in1=rec[:, None, :].to_broadcast([P, C1, H]),
                        op=OP.mult)
                    nc.vector.tensor_tensor(out=h1f[:], in0=h1f[:],
                                            in1=b1_r[:], op=OP.add)
                    # ELU = max(x,0) + exp(min(x,0)) - 1
                    mn = pe.tile([P, D], F32, tag="mn")
                    nc.vector.tensor_tensor(out=mn[:], in0=h1f[:],
                                            in1=zeroD[:], op=OP.min)
                    em = pe.tile([P, D], F32, tag="em")
                    nc.scalar.activation(em[:], mn[:], AF.Exp)
                    nc.vector.tensor_tensor(out=h1f[:], in0=h1f[:],
                                            in1=zeroD[:], op=OP.max)
                    nc.vector.tensor_tensor(out=em[:], in0=em[:],
                                            in1=negoneD[:], op=OP.add)
                    h1b = pg.tile([P, D], BF16, tag="h1b")
                    nc.vector.tensor_tensor(out=h1b[:], in0=h1f[:],
                                            in1=em[:], op=OP.add)
                    # layer-2 node rows [xp2 | als2 | ald2]
                    ps_x2 = pt_ps.tile([P, W2C], F32, tag="ps_x2")
                    for c in range(KD):
                        pt = pt_ps.tile([P, P], BF16, tag="pt")
                        nc.tensor.transpose(pt[:], h1b[:, c * P:(c + 1) * P],
                                            ident[:])
                        cpt = pe.tile([P, P], BF16, tag="cpt")
                        nc.scalar.copy(out=cpt[:], in_=pt[:])
                        nc.tensor.matmul(out=ps_x2[:], lhsT=cpt[:],
                                         rhs=w2sb[:, c, :],
                                         start=(c == 0), stop=(c == KD - 1))
                    nc.scalar.copy(out=xp2_res[:, g, :], in_=ps_x2[:])
                    if g < HL // P:
                        nc.sync.dma_start(out=t2A_loc[w0:w0 + wn, 0:R2],
                                          in_=xp2_res[:wn, g, 0:R2])
                    else:
                        m0 = w0 - HL
                        nc.sync.dma_start(out=t2B_loc[m0:m0 + wn, 0:R2],
                                          in_=xp2_res[:wn, g, 0:R2])
                    if g == HL // P - 1:
                        nc.gpsimd.collective_compute(
                            "AllGather", OP.bypass, replica_groups=rg,
                            ins=[t2A_loc.ap()], outs=[t2A_full.ap()])
                else:
                    smsg = pe.tile([P, D], F32, tag="smsg2")
                    nc.vector.tensor_tensor(
                        out=smsg[:], in0=res[:, g, 0:D],
                        in1=exs_res[:, g, :].to_broadcast([P, D]),
                        op=OP.mult)
                    num = pe.tile([P, D], F32, tag="num2")
                    nc.vector.tensor_tensor(out=num[:], in0=ps_g[:, 0:D],
                                            in1=smsg[:], op=OP.add)
                    x2 = pe.tile([P, D], F32, tag="x2")
                    nc.vector.tensor_tensor(
                        out=x2[:], in0=num[:],
                        in1=rec[:, 0:1].to_broadcast([P, D]), op=OP.mult)
                    nc.vector.tensor_tensor(out=x2[:], in0=x2[:],
                                            in1=b2_r[:], op=OP.add)
                    mx = pe.tile([P, 1], F32, tag="mx")
                    nc.vector.tensor_reduce(out=mx[:], in_=x2[:],
                                            axis=mybir.AxisListType.X,
                                            op=OP.max)
                    nc.vector.tensor_tensor(
                        out=xs_res[:, g, :], in0=x2[:],
                        in1=mx[:, 0:1].to_broadcast([P, D]), op=OP.subtract)
                    es = pe.tile([P, D], F32, tag="es")
                    nc.scalar.activation(es[:], xs_res[:, g, :], AF.Exp,
                                         accum_out=ssum_res[:, g:g + 1])
            if layer == 1:
                nc.gpsimd.collective_compute(
                    "AllGather", OP.bypass, replica_groups=rg,
                    ins=[t2B_loc.ap()], outs=[t2B_full.ap()])
            else:
                # batched log-softmax normalizer + single output write
                ls_all = pres.tile([P, G], F32, tag="ls_all")
                nc.scalar.activation(ls_all[:], ssum_res[:], AF.Ln)
                nc.vector.tensor_tensor(
                    out=xs_res[:], in0=xs_res[:],
                    in1=ls_all[:, :, None].to_broadcast([P, G, D]),
                    op=OP.subtract)
                GF = B // P          # 48 full groups
                nc.sync.dma_start(
                    out=out2[0:GF * P, :].rearrange("(g p) d -> p g d", p=P),
                    in_=xs_res[:, 0:GF, :])
                nc.sync.dma_start(out=out2[GF * P:B, :],
                                  in_=xs_res[0:B - GF * P, GF, :])

        cctx = ExitStack()
        edge_phase(1, cctx)
        cctx.close()
        fctx = ExitStack()
        edge_phase(2, fctx)
        fctx.close()

    nc.compile()
    return nc


def make_in_maps(dims: Dims, inputs: dict, per_core_meta):
    """Per-core input maps. W1/a1*/b1 columns are reordered to the
    (c,h)-interleaved layout the kernel uses internally (pure relayout)."""
    H1, C1, D1 = dims.H1, dims.C1, dims.D1
    perm = np.arange(D1).reshape(H1, C1).T.reshape(-1)   # [h*C+c] -> [c*H+h]
    x = np.asarray(inputs["x"], dtype=np.float32)
    W2 = np.asarray(inputs["W2"], np.float32)
    reps = {
        "W1": np.ascontiguousarray(
            np.asarray(inputs["W1"], np.float32)[:, perm]),
        "a1s": np.ascontiguousarray(
            np.asarray(inputs["a1_src"], np.float32).reshape(-1)[perm]),
        "a1d": np.ascontiguousarray(
            np.asarray(inputs["a1_dst"], np.float32).reshape(-1)[perm]),
        "b1": np.ascontiguousarray(
            np.asarray(inputs["b1"], np.float32).reshape(-1)[perm]),
        "W2": np.ascontiguousarray(W2[perm, :]),
        "a2s": np.asarray(inputs["a2_src"], np.float32).reshape(-1),
        "a2d": np.asarray(inputs["a2_dst"], np.float32).reshape(-1),
        "b2": np.asarray(inputs["b2"], np.float32).reshape(-1),
    }
    in_maps = []
    B = dims.B
    for k in range(dims.NC):
        m = dict(reps)
        m["xT"] = np.ascontiguousarray(x[k * B:(k + 1) * B, :].T)
        m.update(per_core_meta[k])
        in_maps.append(m)
    return in_maps


_CACHE = {}


def _get_program(dims: Dims):
    key = (dims.N, dims.E, dims.NC, tuple(dims.nA), tuple(dims.nB))
    if key not in _CACHE:
        _CACHE[key] = build_program(dims)
    return _CACHE[key]


def kernel(x: np.ndarray, edge_index: np.ndarray, W1, a1_src, a1_dst, b1,
           W2, a2_src, a2_dst, b2) -> np.ndarray:
    x = np.asarray(x)
    edge_index = np.asarray(edge_index)
    dims = Dims(N=x.shape[0], E=edge_index.shape[1], n_cores=8)
    per_core = host_prep(dims, edge_index)
    nc = _get_program(dims)
    in_maps = make_in_maps(
        dims,
        dict(x=x, edge_index=edge_index, W1=W1, a1_src=a1_src, a1_dst=a1_dst,
             b1=b1, W2=W2, a2_src=a2_src, a2_dst=a2_dst, b2=b2),
        per_core)
    res = run_bass_kernel_spmd(nc, in_maps, core_ids=list(range(dims.NC)))
    out = np.concatenate([r["out2"] for r in res.results], axis=0)
    return out.astype(np.float32)


# revision 19
# speedup vs baseline: 1.1044x; 1.1044x over previous
"""2-layer GAT (PyG GATConv style) distributed across 8 TRN2 NeuronCores.

Sharding: nodes partitioned into 8 contiguous destination blocks (one per
core).  The Q7/SWDGE descriptor generation for per-edge dma_gathers is the
machine bottleneck, so the design minimizes gather descriptors:

  - Slot-level edge layout (no Q-slot bins): slot (p, c) of a 128-dst group
    holds exactly one edge; per-(group, half) gather counts are truncated to
    the real edge count (padded to 16), so descriptors ~= edges.
  - Self-loops never enter the gather path: their contribution is applied
    densely in the per-group epilogue from SBUF-resident own-block rows.
  - No per-bin dst-logit gather: ald[dst] is delivered to slots by C small
    TensorE matmuls per group (lhsT = transposed 0/1 dst-selector built on
    DVE from a partition-broadcast flat dst image).
  - Attention logits als/ald are computed by widening the xp matmul rhs with
    precomputed W@a columns (no DVE reductions).

Node tables are bf16 rows [xp | als] gathered by src (768B pitch layer 1,
256B pitch layer 2), split into two sub-tables (local row < HL) so gather
indices fit int16; each sub-table is AllGathered separately so collectives
overlap compute.  Host preprocessing is index-only; all float math runs on
device.
"""

import math
from contextlib import ExitStack

import numpy as np
import ml_dtypes

import concourse.bass as bass
import concourse.tile as tile
from concourse import bacc, mybir
from concourse.bass_utils import run_bass_kernel_spmd
from concourse.masks import make_identity

F32 = mybir.dt.float32
BF16 = mybir.dt.bfloat16
I16 = mybir.dt.int16
AF = mybir.ActivationFunctionType
OP = mybir.AluOpType

P = 128
NEG_SLOPE = 0.2
HL = 3200             # local-row split: half A = rows [0, HL) of each block


def dma_gather_raw(gp, out_ap, in_ap, idxs_ap, num_idxs, elem_size,
                   elem_step, single_packet=None):
    """BassGpSimd.dma_gather minus the payload%256 assert (pitch must still
    be a 256B multiple; verified on HW that arbitrary payload works)."""
    from concourse._compat import exact_div
    assert idxs_ap.dtype == mybir.dt.int16
    assert in_ap.dtype == out_ap.dtype
    stride_bytes = elem_step * mybir.dt.size(in_ap.dtype)
    stride_bytes_256 = exact_div(stride_bytes, 256)
    assert stride_bytes_256 < 256
    _in_ap = gp.lower_ap_dma(in_ap, for_custom_bir_dma=True)
    _idxs_ap = gp.lower_ap(idxs_ap)
    _out_ap = gp.lower_ap(out_ap)
    return gp.add_instruction(
        mybir.InstDMAGatherAnt(
            name=gp.bass.get_next_instruction_name(),
            ins=[*_in_ap, _idxs_ap, gp.lower_val_access(gp.to_reg(num_idxs))],
            outs=[_out_ap],
            transpose=False,
            num_idxs=num_idxs,
            elem_size=elem_size,
            stride_bytes_256=stride_bytes_256,
            gen_mode=0,
            single_packet=(False if single_packet is None
                           else single_packet),
            queue_num=0,
            sbuf_tokens_per_rank=0,
            sbuf_free_dim_per_rank=0,
            sbuf_free_dim_pad_per_rank=0,
            sbuf_byte_offset=0,
        ))


class Dims:
    def __init__(self, N, E, n_cores, H1=4, C1=64, H2=1, C2=16, F_in=256):
        self.N, self.E, self.NC = N, E, n_cores
        self.F_in = F_in
        self.H1, self.C1, self.H2, self.C2 = H1, C1, H2, C2
        self.D1 = H1 * C1          # 256
        self.D2 = H2 * C2          # 16
        self.B = N // n_cores      # 6250
        self.G = math.ceil(self.B / P)   # 49
        self.HLB = self.B - HL     # B-half rows per block (3050)
        self.NA = n_cores * HL     # 25600
        self.NB = n_cores * self.HLB   # 24400
        self.R1 = self.D1 + self.H1    # 260: [xp1 | als1]
        self.T1 = 384                  # layer-1 table pitch (768B bf16)
        self.R2 = self.D2 + self.H2    # 17: [xp2 | als2]
        self.T2 = 128                  # layer-2 table pitch (256B bf16)
        # set by host_prep (shared across cores; per-group capacities)
        self.nA = None   # [G] gather counts half A (mult of 16)
        self.nB = None
        self.CA = None   # [G] column counts half A
        self.CB = None
        self.CMAX = None
        self.SAmax = None  # idx-image cols per group (A)
        self.SBmax = None


def _wrap_idx16(flat, cols):
    """index list -> [128, cols] int16 image (16-partition wrap, replicated
    for the 8 Q7 cores). flat may be shorter than 16*cols (zero pad)."""
    a = np.zeros((16, cols), np.int16)
    n = len(flat)
    i = np.arange(n)
    a[i % 16, i // 16] = flat
    return np.tile(a, (8, 1))


def host_prep(dims: Dims, edge_index: np.ndarray):
    """Index-only preprocessing. Self-loops are NOT added here (handled
    densely on device). Edges are bucketed per (core, dst-group, src-half)
    and assigned slots (p, c) in arrival order: slot i -> (c=i//128,
    p=i%128)."""
    N, NC, B, G = dims.N, dims.NC, dims.B, dims.G
    src = edge_index[0].astype(np.int64)
    dst = edge_index[1].astype(np.int64)

    # per (core, group, half) edge lists (table-local row ids + local dst)
    counts = np.zeros((NC, G, 2), np.int64)
    lists = [[None] * G for _ in range(NC)]
    for k in range(NC):
        m = (dst >= k * B) & (dst < (k + 1) * B)
        s_k = src[m]
        d_k = dst[m] - k * B
        half = ((s_k % B) >= HL).astype(np.int64)
        # table-local row id
        rowA = (s_k // B) * HL + (s_k % B)
        rowB = (s_k // B) * dims.HLB + (s_k % B) - HL
        row = np.where(half == 0, rowA, rowB)
        g = d_k // P
        p_loc = d_k % P
        order = np.lexsort((row, g, half))  # group by (half, g)
        s_o, g_o, h_o, r_o, p_o = (s_k[order], g[order], half[order],
                                   row[order], p_loc[order])
        for gg in range(G):
            for hh in range(2):
                mm = (g_o == gg) & (h_o == hh)
                if lists[k][gg] is None:
                    lists[k][gg] = [None, None]
                lists[k][gg][hh] = (r_o[mm], p_o[mm])
                counts[k, gg, hh] = mm.sum()

    def pad16(x):
        return ((x + 15) // 16) * 16

    nA = pad16(counts[:, :, 0].max(axis=0))
    nB = pad16(counts[:, :, 1].max(axis=0))
    nA = np.maximum(nA, 16)
    nB = np.maximum(nB, 16)
    CA = np.ceil(nA / P).astype(np.int64)
    CB = np.ceil(nB / P).astype(np.int64)
    dims.nA, dims.nB = [int(v) for v in nA], [int(v) for v in nB]
    dims.CA, dims.CB = [int(v) for v in CA], [int(v) for v in CB]
    dims.CMAX = int((CA + CB).max())
    dims.SAmax = int(nA.max() // 16)
    dims.SBmax = int(nB.max() // 16)

    CMAX = dims.CMAX
    SW = dims.SAmax + dims.SBmax
    per_core = []
    for k in range(NC):
        idx = np.zeros((P, G * SW), np.int16)
        dstl = np.full((P, G * CMAX), -1.0, np.float32)
        dstlF = np.full((G, CMAX * P), -1.0, np.float32)
        for g in range(G):
            C_g = dims.CA[g] + dims.CB[g]
            dflat = np.full(C_g * P, -1.0, np.float32)
            for hh in range(2):
                rows, ps = lists[k][g][hh]
                n = len(rows)
                coff = 0 if hh == 0 else dims.CA[g]
                i = np.arange(n)
                c = i // P + coff
                p = i % P
                # slot (p, c) holds an edge whose LOCAL dst (d % 128) is ps
                dstl[p, g * CMAX + c] = ps
                dflat[c * P + p] = ps
                cap = dims.nA[g] if hh == 0 else dims.nB[g]
                scols = dims.SAmax if hh == 0 else dims.SBmax
                soff = g * SW + (0 if hh == 0 else dims.SAmax)
                idx[:, soff:soff + scols] = _wrap_idx16(rows, scols)
            dstlF[g, 0:C_g * P] = dflat
        per_core.append(dict(
            idx=idx,
            dstl=np.ascontiguousarray(dstl).astype(ml_dtypes.bfloat16),
            dstlF=np.ascontiguousarray(dstlF).astype(ml_dtypes.bfloat16),
        ))
    return per_core


def build_program(dims: Dims):
    N, NC, B, G = dims.N, dims.NC, dims.B, dims.G
    F_in, D1, D2, H1, H2 = dims.F_in, dims.D1, dims.D2, dims.H1, dims.H2
    C1 = dims.C1
    R1, T1, R2, T2 = dims.R1, dims.T1, dims.R2, dims.T2
    NA, NB, HLB = dims.NA, dims.NB, dims.HLB
    CMAX = dims.CMAX
    SW = dims.SAmax + dims.SBmax
    KF = F_in // P     # 2
    KD = D1 // P       # 2
    W1C = D1 + 2 * H1  # w1 rhs width: [W1 | va1s | va1d] = 264
    W2C = D2 + 2 * H2  # 18

    nc = bacc.Bacc("TRN2", target_bir_lowering=False, debug=False,
                   enable_asserts=False, num_devices=NC)

    xT = nc.dram_tensor("xT", [F_in, B], F32, kind="ExternalInput")
    W1 = nc.dram_tensor("W1", [F_in, D1], F32, kind="ExternalInput")
    a1s = nc.dram_tensor("a1s", [D1], F32, kind="ExternalInput")
    a1d = nc.dram_tensor("a1d", [D1], F32, kind="ExternalInput")
    b1 = nc.dram_tensor("b1", [D1], F32, kind="ExternalInput")
    W2 = nc.dram_tensor("W2", [D1, D2], F32, kind="ExternalInput")
    a2s = nc.dram_tensor("a2s", [D2], F32, kind="ExternalInput")
    a2d = nc.dram_tensor("a2d", [D2], F32, kind="ExternalInput")
    b2 = nc.dram_tensor("b2", [D2], F32, kind="ExternalInput")
    idx = nc.dram_tensor("idx", [P, G * SW], I16, kind="ExternalInput")
    dstl = nc.dram_tensor("dstl", [P, G * CMAX], BF16, kind="ExternalInput")
    dstlF = nc.dram_tensor("dstlF", [G, CMAX * P], BF16,
                           kind="ExternalInput")
    out2 = nc.dram_tensor("out2", [B, D2], F32, kind="ExternalOutput")

    t1A_loc = nc.dram_tensor("t1A_loc", [HL, T1], BF16)
    t1B_loc = nc.dram_tensor("t1B_loc", [HLB, T1], BF16)
    t1A_full = nc.dram_tensor("t1A_full", [NA, T1], BF16, addr_space="Shared")
    t1B_full = nc.dram_tensor("t1B_full", [NB, T1], BF16, addr_space="Shared")
    t2A_loc = nc.dram_tensor("t2A_loc", [HL, T2], BF16)
    t2B_loc = nc.dram_tensor("t2B_loc", [HLB, T2], BF16)
    t2A_full = nc.dram_tensor("t2A_full", [NA, T2], BF16, addr_space="Shared")
    t2B_full = nc.dram_tensor("t2B_full", [NB, T2], BF16, addr_space="Shared")

    rg = [list(range(NC))]

    with tile.TileContext(nc) as tc, ExitStack() as ctx:
        const = ctx.enter_context(tc.tile_pool(name="const", bufs=1))
        ictx = ExitStack()
        cpsum = ictx.enter_context(tc.tile_pool(name="cpsum", bufs=1,
                                                space="PSUM"))
        itmp = ictx.enter_context(tc.tile_pool(name="itmp", bufs=1))

        iota_i = itmp.tile([P, P], mybir.dt.int32, tag="iota_i")
        nc.gpsimd.iota(iota_i[:], pattern=[[1, P]], base=0,
                       channel_multiplier=0)
        iota_bf = const.tile([P, P], BF16, tag="iota_bf")
        nc.vector.tensor_copy(iota_bf[:], iota_i[:])
        # partition-index iota, constant along free dim
        iotaP_i = itmp.tile([P, CMAX * P], mybir.dt.int32, tag="iotaP_i")
        nc.gpsimd.iota(iotaP_i[:], pattern=[[0, CMAX * P]], base=0,
                       channel_multiplier=1)
        iotaP = const.tile([P, CMAX * P], BF16, tag="iotaP")
        nc.vector.tensor_copy(iotaP[:], iotaP_i[:])
        ident = const.tile([P, P], BF16, tag="ident")
        make_identity(nc, ident[:])

        # weights rhs: [W1 | va1s | va1d], [W2 | va2s | va2d]
        w1sb = const.tile([P, KF, W1C], BF16, tag="w1sb")
        for c in range(KF):
            nc.gpsimd.dma_start(out=w1sb[:, c, 0:D1],
                                in_=W1[c * P:(c + 1) * P, :])
        w2sb = const.tile([P, KD, W2C], BF16, tag="w2sb")
        for c in range(KD):
            nc.gpsimd.dma_start(out=w2sb[:, c, 0:D2],
                                in_=W2[c * P:(c + 1) * P, :])

        ones_row = const.tile([1, P], BF16, tag="ones_row")
        nc.vector.memset(ones_row[:], 1.0)
        ones_rowf = itmp.tile([1, P], F32, tag="ones_rowf")
        nc.vector.memset(ones_rowf[:], 1.0)

        def replicate(vec_ap, X, tag, pool):
            vrow = itmp.tile([1, X], F32, tag=tag + "_row")
            nc.sync.dma_start(out=vrow[:], in_=vec_ap[None, :])
            pr = cpsum.tile([P, X], F32, tag="reppsum")
            nc.tensor.matmul(out=pr[:], lhsT=ones_rowf[:], rhs=vrow[:],
                             start=True, stop=True)
            rep = pool.tile([P, X], F32, tag=tag)
            nc.vector.tensor_copy(rep[:], pr[:])
            return rep

        a1s_r = replicate(a1s, D1, "a1s_r", itmp)
        a1d_r = replicate(a1d, D1, "a1d_r", itmp)
        b1_r = replicate(b1, D1, "b1_r", const)
        a2s_r = replicate(a2s, D2, "a2s_r", itmp)
        a2d_r = replicate(a2d, D2, "a2d_r", itmp)
        b2_r = replicate(b2, D2, "b2_r", const)

        slopeH = const.tile([P, H1], F32, tag="slopeH")
        nc.vector.memset(slopeH[:], NEG_SLOPE)
        zeroD = const.tile([P, D1], F32, tag="zeroD")
        nc.vector.memset(zeroD[:], 0.0)
        negoneD = const.tile([P, D1], F32, tag="negoneD")
        nc.vector.memset(negoneD[:], -1.0)

        # va1s[f, h] = sum_c W1[f, (c,h)] * a1s[(c,h)]   (features are
        # (c,h)-interleaved: head h at stride-H1 positions)
        for c in range(KF):
            for (vec, off) in ((a1s_r, D1), (a1d_r, D1 + H1)):
                tmpv = itmp.tile([P, D1], F32, tag="vamul")
                nc.vector.tensor_tensor(out=tmpv[:], in0=w1sb[:, c, 0:D1],
                                        in1=vec[:], op=OP.mult)
                vaf = itmp.tile([P, H1], F32, tag="vaf")
                nc.vector.tensor_reduce(
                    out=vaf[:],
                    in_=tmpv[:].rearrange("p (c h) -> p h c", h=H1),
                    axis=mybir.AxisListType.X, op=OP.add)
                nc.vector.tensor_copy(w1sb[:, c, off:off + H1], vaf[:])
        for c in range(KD):
            for (vec, off) in ((a2s_r, D2), (a2d_r, D2 + H2)):
                tmpv = itmp.tile([P, D2], F32, tag="vamul2")
                nc.vector.tensor_tensor(out=tmpv[:], in0=w2sb[:, c, 0:D2],
                                        in1=vec[:], op=OP.mult)
                vaf = itmp.tile([P, H2], F32, tag="vaf2")
                nc.vector.tensor_reduce(out=vaf[:], in_=tmpv[:],
                                        axis=mybir.AxisListType.X, op=OP.add)
                nc.vector.tensor_copy(w2sb[:, c, off:off + H2], vaf[:])

        idx_sb = const.tile([P, G * SW], I16, tag="idx_sb")
        nc.sync.dma_start(out=idx_sb[:], in_=idx[:, :])
        dstl_sb = const.tile([P, G * CMAX], BF16, tag="dstl_sb")
        nc.sync.dma_start(out=dstl_sb[:], in_=dstl[:, :])

        # SBUF-resident per-node rows of the own block:
        # layer 1: [xp1 (256) | als1 (4) | ald1 (4)]; layer 2: [xp2|als2|ald2]
        xp1_res = const.tile([P, G, W1C], BF16, tag="xp1_res")
        nc.vector.memset(xp1_res[:], 0.0)
        xp2_res = const.tile([P, G, W2C], BF16, tag="xp2_res")

        ictx.close()

        # ---- stage A: xp1 + logits for own block; fill t1 tables ----------
        actx = ExitStack()
        pa = actx.enter_context(tc.tile_pool(name="pa", bufs=3))
        pa_ps = actx.enter_context(tc.tile_pool(name="pa_ps", bufs=2,
                                                space="PSUM"))
        # whole x^T block resident in SBUF for stage A, loaded in chunks so
        # the first tiles' matmuls start early (sync f32 load + DVE cast)
        xpool = actx.enter_context(tc.tile_pool(name="xsb", bufs=1))
        xsb = xpool.tile([P, KF, B], BF16, tag="xsb")
        XCH = 8
        xbnd = [round(B * i / XCH) for i in range(XCH + 1)]
        for i in range(XCH):
            lo, hi = xbnd[i], xbnd[i + 1]
            xf = pa.tile([P, KF, (B + XCH - 1) // XCH + 1], F32, tag="xf")
            for c in range(KF):
                nc.sync.dma_start(out=xf[:, c, 0:hi - lo],
                                  in_=xT[c * P:(c + 1) * P, lo:hi])
            nc.vector.tensor_copy(xsb[:, :, lo:hi], xf[:, :, 0:hi - lo])
        for t in range(G):
            n0 = t * P
            nn = min(P, B - n0)
            ps_a = pa_ps.tile([P, W1C], F32, tag="ps_a")
            for c in range(KF):
                nc.tensor.matmul(out=ps_a[:nn, :], lhsT=xsb[:, c, n0:n0 + nn],
                                 rhs=w1sb[:, c, :],
                                 start=(c == 0), stop=(c == KF - 1))
            nc.scalar.copy(out=xp1_res[:nn, t, :], in_=ps_a[:nn, :])
            if t < HL // P:
                nc.sync.dma_start(out=t1A_loc[n0:n0 + nn, 0:R1],
                                  in_=xp1_res[:nn, t, 0:R1])
            else:
                m0 = n0 - HL
                nc.sync.dma_start(out=t1B_loc[m0:m0 + nn, 0:R1],
                                  in_=xp1_res[:nn, t, 0:R1])
            if t == HL // P - 1:
                nc.gpsimd.collective_compute(
                    "AllGather", OP.bypass, replica_groups=rg,
                    ins=[t1A_loc.ap()], outs=[t1A_full.ap()])
        nc.gpsimd.collective_compute(
            "AllGather", OP.bypass, replica_groups=rg,
            ins=[t1B_loc.ap()], outs=[t1B_full.ap()])
        actx.close()

        # ---- edge phase (shared structure for both layers) ----------------
        def edge_phase(layer, fctx):
            """layer 1: gathers t1 rows, aggregates, writes t2 tables.
            layer 2: gathers t2 rows, aggregates, writes log_softmax out."""
            R = R1 if layer == 1 else R2
            D = D1 if layer == 1 else D2
            H = H1 if layer == 1 else H2
            T = T1 if layer == 1 else T2
            tA = t1A_full if layer == 1 else t2A_full
            tB = t1B_full if layer == 1 else t2B_full
            res = xp1_res if layer == 1 else xp2_res

            PRE = 3
            pg = fctx.enter_context(tc.tile_pool(name=f"pg{layer}",
                                                 bufs=PRE + 1))
            pm = fctx.enter_context(tc.tile_pool(name=f"pm{layer}", bufs=2))
            pe = fctx.enter_context(tc.tile_pool(name=f"pe{layer}", bufs=3))
            pres = fctx.enter_context(tc.tile_pool(name=f"pres{layer}",
                                                   bufs=1))
            ps_pool = fctx.enter_context(tc.tile_pool(
                name=f"ps{layer}", bufs=2, space="PSUM"))
            psa_pool = fctx.enter_context(tc.tile_pool(
                name=f"psa{layer}", bufs=2, space="PSUM"))
            psr_pool = fctx.enter_context(tc.tile_pool(
                name=f"psr{layer}", bufs=2, space="PSUM"))
            if layer == 1:
                pt_ps = fctx.enter_context(tc.tile_pool(
                    name="pt_ps", bufs=1, space="PSUM"))

            # batched self-loop softmax numerators: exs_res[p, g, h]
            eps_all = pres.tile([P, G, H], F32, tag="eps_all")
            nc.vector.tensor_tensor(out=eps_all[:], in0=res[:, :, D:D + H],
                                    in1=res[:, :, D + H:D + 2 * H],
                                    op=OP.add)
            lr_all = pres.tile([P, G, H], F32, tag="lr_all")
            nc.vector.tensor_tensor(
                out=lr_all[:], in0=eps_all[:],
                in1=slopeH[:, None, 0:H].to_broadcast([P, G, H]), op=OP.mult)
            nc.vector.tensor_tensor(out=lr_all[:], in0=lr_all[:],
                                    in1=eps_all[:], op=OP.max)
            exs_res = pres.tile([P, G, H], F32, tag="exs_res")
            nc.scalar.activation(exs_res[:], lr_all[:], AF.Exp)
            if layer == 2:
                xs_res = pres.tile([P, G, D], F32, tag="xs_res")
                ssum_res = pres.tile([P, G], F32, tag="ssum_res")

            gat_tiles = {}

            def issue_A(g):
                CAg = dims.CA[g]
                nAg = dims.nA[g]
                gat = pg.tile([P, CMAX, R], BF16, tag="gat")
                if g <= PRE:
                    nc.vector.memset(gat[:], 0.0)
                dma_gather_raw(nc.gpsimd, gat[:, 0:CAg, :], tA[0:NA, 0:R],
                               idx_sb[:, g * SW:g * SW + nAg // 16],
                               nAg, R, T)
                gat_tiles[g] = gat

            def issue_B(g):
                CAg, CBg = dims.CA[g], dims.CB[g]
                nBg = dims.nB[g]
                gat = gat_tiles[g]
                dma_gather_raw(nc.gpsimd, gat[:, CAg:CAg + CBg, :],
                               tB[0:NB, 0:R],
                               idx_sb[:, g * SW + dims.SAmax:
                                      g * SW + dims.SAmax + nBg // 16],
                               nBg, R, T)

            for g in range(min(PRE, G)):
                issue_A(g)
            for g in range(min(PRE, G)):
                issue_B(g)

            for g in range(G):
                if g + PRE < G:
                    issue_A(g + PRE)
                    issue_B(g + PRE)
                w0 = g * P
                wn = min(P, B - w0)
                CAg, CBg = dims.CA[g], dims.CB[g]
                Cg = CAg + CBg
                gat = gat_tiles.pop(g)

                # transposed selector stT[d, c*128+p] = (dst(p,c) == d):
                # replicate the flat dst row across partitions via TensorE,
                # then one DVE compare against the partition-index iota.
                flrow = pe.tile([1, CMAX * P], BF16, tag="flrow")
                nc.sync.dma_start(out=flrow[0:1, 0:Cg * P],
                                  in_=dstlF[g:g + 1, 0:Cg * P])
                fl = flrow[0:1, 0:Cg * P]
                stT = pm.tile([P, CMAX * P], BF16, tag="stT")
                for o in range(0, Cg * P, 512):
                    w = min(512, Cg * P - o)
                    rep = psr_pool.tile([P, 512], F32, tag="rep")
                    nc.tensor.matmul(out=rep[:, 0:w], lhsT=ones_row[:],
                                     rhs=fl[:, o:o + w],
                                     start=True, stop=True)
                    nc.vector.tensor_tensor(
                        out=stT[:, o:o + w], in0=iotaP[:, o:o + w],
                        in1=rep[:, 0:w], op=OP.is_equal)
                # ald per slot via C small matmuls
                ps_ald = psa_pool.tile([P, CMAX * H], F32, tag="ps_ald")
                for c in range(Cg):
                    nc.tensor.matmul(
                        out=ps_ald[:, c * H:(c + 1) * H],
                        lhsT=stT[:, c * P:(c + 1) * P],
                        rhs=res[:, g, D + H:D + 2 * H],
                        start=True, stop=True)

                # ex = exp(leaky_relu(als[s] + ald[d]))
                ep = pe.tile([P, CMAX, H], F32, tag="ep")
                nc.vector.tensor_tensor(
                    out=ep[:, 0:Cg, :], in0=gat[:, 0:Cg, D:D + H],
                    in1=ps_ald[:].rearrange("p (c h) -> p c h",
                                            h=H)[:, 0:Cg, :],
                    op=OP.add)
                lr = pe.tile([P, CMAX, H], F32, tag="lr")
                nc.vector.tensor_tensor(
                    out=lr[:, 0:Cg, :], in0=ep[:, 0:Cg, :],
                    in1=slopeH[:, None, 0:H].to_broadcast([P, Cg, H]),
                    op=OP.mult)
                nc.vector.tensor_tensor(out=lr[:, 0:Cg, :],
                                        in0=lr[:, 0:Cg, :],
                                        in1=ep[:, 0:Cg, :], op=OP.max)
                msg = pm.tile([P, CMAX, R], BF16, tag="msg")
                nc.scalar.activation(msg[:, 0:Cg, D:D + H], lr[:, 0:Cg, :],
                                     AF.Exp)
                if layer == 1:
                    nc.vector.tensor_tensor(
                        out=msg[:, 0:Cg, 0:D].rearrange(
                            "p k (c h) -> p k c h", h=H),
                        in0=gat[:, 0:Cg, 0:D].rearrange(
                            "p k (c h) -> p k c h", h=H),
                        in1=msg[:, 0:Cg, D:D + H][:, :, None, :].to_broadcast(
                            [P, Cg, C1, H]),
                        op=OP.mult)
                else:
                    nc.vector.tensor_tensor(
                        out=msg[:, 0:Cg, 0:D], in0=gat[:, 0:Cg, 0:D],
                        in1=msg[:, 0:Cg, D:D + H].to_broadcast([P, Cg, D]),
                        op=OP.mult)

                # segment-sum matmuls: selector st[p, c, d]
                st = pm.tile([P, CMAX, P], BF16, tag="st")
                nc.vector.tensor_tensor(
                    out=st[:, 0:Cg, :],
                    in0=iota_bf[:, None, :].to_broadcast([P, Cg, P]),
                    in1=dstl_sb[:, g * CMAX:g * CMAX + Cg][:, :, None]
                        .to_broadcast([P, Cg, P]),
                    op=OP.is_equal)
                ps_g = ps_pool.tile([P, R], F32, tag="ps_g")
                for col in range(Cg):
                    nc.tensor.matmul(out=ps_g[:], lhsT=st[:, col, :],
                                     rhs=msg[:, col, :],
                                     start=(col == 0), stop=(col == Cg - 1))

                # epilogue: self-loop + alpha-normalize
                den = pe.tile([P, H], F32, tag="den")
                nc.vector.tensor_tensor(out=den[:], in0=ps_g[:, D:D + H],
                                        in1=exs_res[:, g, :], op=OP.add)
                rec = pe.tile([P, H], F32, tag="rec")
                nc.vector.reciprocal(rec[:], den[:])

                if layer == 1:
                    smsg = pe.tile([P, D], F32, tag="smsg")
                    nc.vector.tensor_tensor(
                        out=smsg[:].rearrange("p (c h) -> p c h", h=H),
                        in0=res[:, g, 0:D].rearrange("p (c h) -> p c h", h=H),
                        in1=exs_res[:, g, :][:, None, :].to_broadcast(
                            [P, C1, H]),
                        op=OP.mult)
                    num = pe.tile([P, D], F32, tag="num")
                    nc.vector.tensor_tensor(out=num[:], in0=ps_g[:, 0:D],
                                            in1=smsg[:], op=OP.add)
                    h1f = pg.tile([P, D], F32, tag="h1f")
                    nc.vector.tensor_tensor(
                        out=h1f[:].rearrange("p (c h) -> p c h", h=H),
                        in0=num[:].rearrange("p (c h) -> p c h", h=H),
                        in1=rec[:, None, :].to_broadcast([P, C1, H]),
                        op=OP.mult)
                    nc.vector.tensor_tensor(out=h1f[:], in0=h1f[:],
                                            in1=b1_r[:], op=OP.add)
                    # ELU = max(x,0) + exp(min(x,0)) - 1
                    mn = pe.tile([P, D], F32, tag="mn")
                    nc.vector.tensor_tensor(out=mn[:], in0=h1f[:],
                                            in1=zeroD[:], op=OP.min)
                    em = pe.tile([P, D], F32, tag="em")
                    nc.scalar.activation(em[:], mn[:], AF.Exp)
                    nc.vector.tensor_tensor(out=h1f[:], in0=h1f[:],
                                            in1=zeroD[:], op=OP.max)
                    nc.vector.tensor_tensor(out=em[:], in0=em[:],
                                            in1=negoneD[:], op=OP.add)
                    h1b = pg.tile([P, D], BF16, tag="h1b")
                    nc.vector.tensor_tensor(out=h1b[:], in0=h1f[:],
                                            in1=em[:], op=OP.add)
                    # layer-2 node rows [xp2 | als2 | ald2]
                    ps_x2 = pt_ps.tile([P, W2C], F32, tag="ps_x2")
                    for c in range(KD):
                        pt = pt_ps.tile([P, P], BF16, tag="pt")
                        nc.tensor.transpose(pt[:], h1b[:, c * P:(c + 1) * P],
                                            ident[:])
                        cpt = pe.tile([P, P], BF16, tag="cpt")
                        nc.scalar.copy(out=cpt[:], in_=pt[:])
                        nc.tensor.matmul(out=ps_x2[:], lhsT=cpt[:],
                                         rhs=w2sb[:, c, :],
                                         start=(c == 0), stop=(c == KD - 1))
                    nc.scalar.copy(out=xp2_res[:, g, :], in_=ps_x2[:])
                    if g < HL // P:
                        nc.sync.dma_start(out=t2A_loc[w0:w0 + wn, 0:R2],
                                          in_=xp2_res[:wn, g, 0:R2])
                    else:
                        m0 = w0 - HL
                        nc.sync.dma_start(out=t2B_loc[m0:m0 + wn, 0:R2],
                                          in_=xp2_res[:wn, g, 0:R2])
                    if g == HL // P - 1:
                        nc.gpsimd.collective_compute(
                            "AllGather", OP.bypass, replica_groups=rg,
                            ins=[t2A_loc.ap()], outs=[t2A_full.ap()])
                else:
                    smsg = pe.tile([P, D], F32, tag="smsg2")
                    nc.vector.tensor_tensor(
                        out=smsg[:], in0=res[:, g, 0:D],
                        in1=exs_res[:, g, :].to_broadcast([P, D]),
                        op=OP.mult)
                    num = pe.tile([P, D], F32, tag="num2")
                    nc.vector.tensor_tensor(out=num[:], in0=ps_g[:, 0:D],
                                            in1=smsg[:], op=OP.add)
                    x2 = pe.tile([P, D], F32, tag="x2")
                    nc.vector.tensor_tensor(
                        out=x2[:], in0=num[:],
                        in1=rec[:, 0:1].to_broadcast([P, D]), op=OP.mult)
                    nc.vector.tensor_tensor(out=x2[:], in0=x2[:],
                                            in1=b2_r[:], op=OP.add)
                    mx = pe.tile([P, 1], F32, tag="mx")
                    nc.vector.tensor_reduce(out=mx[:], in_=x2[:],
                                            axis=mybir.AxisListType.X,
                                            op=OP.max)
                    nc.vector.tensor_tensor(
                        out=xs_res[:, g, :], in0=x2[:],
                        in1=mx[:, 0:1].to_broadcast([P, D]), op=OP.subtract)
                    es = pe.tile([P, D], F32, tag="es")
                    nc.scalar.activation(es[:], xs_res[:, g, :], AF.Exp,
                                         accum_out=ssum_res[:, g:g + 1])
            if layer == 1:
                nc.gpsimd.collective_compute(
                    "AllGather", OP.bypass, replica_groups=rg,
                    ins=[t2B_loc.ap()], outs=[t2B_full.ap()])
            else:
                # batched log-softmax normalizer + single output write
                ls_all = pres.tile([P, G], F32, tag="ls_all")
                nc.scalar.activation(ls_all[:], ssum_res[:], AF.Ln)
                nc.vector.tensor_tensor(
                    out=xs_res[:], in0=xs_res[:],
                    in1=ls_all[:, :, None].to_broadcast([P, G, D]),
                    op=OP.subtract)
                GF = B // P          # 48 full groups
                nc.sync.dma_start(
                    out=out2[0:GF * P, :].rearrange("(g p) d -> p g d", p=P),
                    in_=xs_res[:, 0:GF, :])
                nc.sync.dma_start(out=out2[GF * P:B, :],
                                  in_=xs_res[0:B - GF * P, GF, :])

        cctx = ExitStack()
        edge_phase(1, cctx)
        cctx.close()
        fctx = ExitStack()
        edge_phase(2, fctx)
        fctx.close()

    nc.compile()
    return nc


def make_in_maps(dims: Dims, inputs: dict, per_core_meta):
    """Per-core input maps. W1/a1*/b1 columns are reordered to the
    (c,h)-interleaved layout the kernel uses internally (pure relayout)."""
    H1, C1, D1 = dims.H1, dims.C1, dims.D1
    perm = np.arange(D1).reshape(H1, C1).T.reshape(-1)   # [h*C+c] -> [c*H+h]
    x = np.asarray(inputs["x"], dtype=np.float32)
    W2 = np.asarray(inputs["W2"], np.float32)
    reps = {
        "W1": np.ascontiguousarray(
            np.asarray(inputs["W1"], np.float32)[:, perm]),
        "a1s": np.ascontiguousarray(
            np.asarray(inputs["a1_src"], np.float32).reshape(-1)[perm]),
        "a1d": np.ascontiguousarray(
            np.asarray(inputs["a1_dst"], np.float32).reshape(-1)[perm]),
        "b1": np.ascontiguousarray(
            np.asarray(inputs["b1"], np.float32).reshape(-1)[perm]),
        "W2": np.ascontiguousarray(W2[perm, :]),
        "a2s": np.asarray(inputs["a2_src"], np.float32).reshape(-1),
        "a2d": np.asarray(inputs["a2_dst"], np.float32).reshape(-1),
        "b2": np.asarray(inputs["b2"], np.float32).reshape(-1),
    }
    in_maps = []
    B = dims.B
    for k in range(dims.NC):
        m = dict(reps)
        m["xT"] = np.ascontiguousarray(x[k * B:(k + 1) * B, :].T)
        m.update(per_core_meta[k])
        in_maps.append(m)
    return in_maps


_CACHE = {}


def _get_program(dims: Dims):
    key = (dims.N, dims.E, dims.NC, tuple(dims.nA), tuple(dims.nB))
    if key not in _CACHE:
        _CACHE[key] = build_program(dims)
    return _CACHE[key]


def kernel(x: np.ndarray, edge_index: np.ndarray, W1, a1_src, a1_dst, b1,
           W2, a2_src, a2_dst, b2) -> np.ndarray:
    x = np.asarray(x)
    edge_index = np.asarray(edge_index)
    dims = Dims(N=x.shape[0], E=edge_index.shape[1], n_cores=8)
    per_core = host_prep(dims, edge_index)
    nc = _get_program(dims)
    in_maps = make_in_maps(
        dims,
        dict(x=x, edge_index=edge_index, W1=W1, a1_src=a1_src, a1_dst=a1_dst,
             b1=b1, W2=W2, a2_src=a2_src, a2_dst=a2_dst, b2=b2),
        per_core)
    res = run_bass_kernel_spmd(nc, in_maps, core_ids=list(range(dims.NC)))
    out = np.concatenate([r["out2"] for r in res.results], axis=0)
    return out.astype(np.float32)
